# revision 1
# baseline (speedup 1.0000x reference)
"""Deformable Conv1d (B=4, C=256, L=8192, K=3, DG=4) on 8 Trainium2 cores.

Sharding: core = (sample b = core//2, L-half h = core%2); each core computes
out[b, :, h*4096:(h+1)*4096] from a haloed window of x[b].

Per-core pipeline:
  conv (PE, bf16): offset/mask convs as shifted-rhs matmuls + an iota matmul
    so PSUM holds p = off + l + (k-1) + HALO after the per-row drain bias.
  small chain (DVE, packed [96,512]): t = p mod 1, a0=(1-t)*m, a1=t*m,
    idx = int16(p - t) clamped.
  dma_gather (transpose=True) from per-dgroup transposed pair tables:
    row e = [x[c, e] for c in dgroup] ++ [x[c, e+1] for c]  (256B bf16 rows).
  A-broadcast (PE selector matmul): replicates a0 to partitions 0-63 and a1
    to 64-127, reading the a-plane through a sigma_c-permuted rhs AP.
  modulate (DVE): M = G * A;  S[64*(dk%2)+...] = M[0:64] + M[64:128].
  main matmul (PE): out = W2 @ S + bias with sigma_c-unpermuting rhs AP.
"""
import sys
sys.path.insert(0, '/opt/trn_rl_repo')
from contextlib import ExitStack
import numpy as np
import ml_dtypes

import concourse.bass as bass
import concourse.tile as tile
from concourse import bacc, mybir

dt = mybir.dt
bf16 = ml_dtypes.bfloat16

B, C, L = 4, 256, 8192
N_CORES = 8
LH = L // 2
HALO = 17
W = LH + 2 * HALO          # 4130 window positions
WROWS = 33 * 128           # 4224 padded rows in pair tables
NCHUNK = 8
CH = 512
NBATCH = 4                 # 2-chunk modulate batches
BCH = 2 * CH
AF = mybir.ActivationFunctionType
ALU = mybir.AluOpType


def build_program(n_reps=1):
    nc = bacc.Bacc("TRN2", target_bir_lowering=False, debug=False,
                   enable_asserts=True, num_devices=N_CORES,
                   num_swdge_queues=2, dynamic_dma_scratch_size=24576)

    def din(name, shape, dty):
        return nc.dram_tensor(name, shape, dty, kind="ExternalInput").ap()

    xT = din("xT", (4, 128, WROWS), dt.bfloat16)
    xP = din("xP", (2, 128, W), dt.bfloat16)
    wconv = din("wconv", (2, 3, 128, 44), dt.bfloat16)
    iotas = din("iotas", (2, 128, CH), dt.float32)
    boff = din("boff", (12, 1), dt.float32)
    bmask = din("bmask", (12, 1), dt.float32)
    wmain = din("wmain", (6, 2, 128, 128), dt.bfloat16)
    bmain = din("bmain", (2, 128, 1), dt.float32)
    wsel = din("wsel", (12, 24, 128), dt.bfloat16)
    yout = nc.dram_tensor("y", (2, 128, LH), dt.float32, kind="ExternalOutput").ap()

    with ExitStack() as ctx:
        tc = ctx.enter_context(tile.TileContext(nc))
        cpool = ctx.enter_context(tc.tile_pool(name="const", bufs=1))
        ppool = ctx.enter_context(tc.tile_pool(name="planes", bufs=1))
        gpool = ctx.enter_context(tc.tile_pool(name="g", bufs=2))
        mpool = ctx.enter_context(tc.tile_pool(name="mtp", bufs=2))
        apool = ctx.enter_context(tc.tile_pool(name="a", bufs=1))
        spool = ctx.enter_context(tc.tile_pool(name="s", bufs=1))
        opool = ctx.enter_context(tc.tile_pool(name="o", bufs=2))
        psc = ctx.enter_context(tc.tile_pool(name="psc", bufs=2, space="PSUM"))
        psb = ctx.enter_context(tc.tile_pool(name="psb", bufs=2, space="PSUM"))
        psm = ctx.enter_context(tc.tile_pool(name="psm", bufs=2, space="PSUM"))

        t_xT = [cpool.tile([128, WROWS], dt.bfloat16, tag=f"xT{d}", name=f"xT{d}") for d in range(4)]
        for d in range(4):
            nc.sync.dma_start(t_xT[d][:], xT[d])
        t_xP = [cpool.tile([128, W], dt.bfloat16, tag=f"xP{cb}", name=f"xP{cb}") for cb in range(2)]
        for cb in range(2):
            nc.sync.dma_start(t_xP[cb][:], xP[cb])
        t_wconv = [[cpool.tile([128, 44], dt.bfloat16, tag=f"wc{cb}{k}", name=f"wc{cb}{k}")
                    for k in range(3)] for cb in range(2)]
        for cb in range(2):
            for k in range(3):
                nc.sync.dma_start(t_wconv[cb][k][:], wconv[cb, k])
        t_iot = [cpool.tile([128, CH], dt.float32, tag=f"iot{t}", name=f"iot{t}")
                 for t in range(2)]
        for t in range(2):
            nc.sync.dma_start(t_iot[t][:], iotas[t])
        t_boff = cpool.tile([12, 1], dt.float32, name="boff")
        nc.sync.dma_start(t_boff[:], boff[:])
        t_bmask = cpool.tile([12, 1], dt.float32, name="bmask")
        nc.sync.dma_start(t_bmask[:], bmask[:])
        t_wmain = [[cpool.tile([128, 128], dt.bfloat16, tag=f"wm{kb}{mb}", name=f"wm{kb}{mb}")
                    for mb in range(2)] for kb in range(6)]
        for kb in range(6):
            for mb in range(2):
                nc.sync.dma_start(t_wmain[kb][mb][:], wmain[kb, mb])
        t_bmain = [cpool.tile([128, 1], dt.float32, tag=f"bm{mb}", name=f"bm{mb}") for mb in range(2)]
        for mb in range(2):
            nc.sync.dma_start(t_bmain[mb][:], bmain[mb])
        t_wsel = [cpool.tile([24, 128], dt.bfloat16, tag=f"sel{dk}", name=f"sel{dk}") for dk in range(12)]
        for dk in range(12):
            nc.sync.dma_start(t_wsel[dk][:], wsel[dk])

        for _rep in range(n_reps):
            pk = [ppool.tile([128, CH], dt.float32, tag=f"pk{t}", name=f"pk{t}") for t in range(2)]
            mk = [ppool.tile([128, CH], dt.float32, tag=f"mk{t}", name=f"mk{t}") for t in range(2)]
            ttl = [ppool.tile([128, CH], dt.float32, tag=f"ttl{t}", name=f"ttl{t}") for t in range(2)]
            p0f = [ppool.tile([128, CH], dt.float32, tag=f"p0f{t}", name=f"p0f{t}") for t in range(2)]
            a0p = [ppool.tile([128, CH], dt.bfloat16, tag=f"a0p{t}", name=f"a0p{t}") for t in range(2)]
            a1p = [ppool.tile([128, CH], dt.bfloat16, tag=f"a1p{t}", name=f"a1p{t}") for t in range(2)]
            i16p = [ppool.tile([128, CH], dt.int16, tag=f"i16p{t}", name=f"i16p{t}") for t in range(2)]
            aplane = ppool.tile([24, LH], dt.bfloat16, tag="aplane", name="aplane")
            idxt = [ppool.tile([128, LH // 16], dt.int16, tag=f"idx{dk}", name=f"idx{dk}")
                    for dk in range(12)]

            # ---- conv + drains --------------------------------------------
            for c in range(NCHUNK):
                ps = psc.tile([64, CH], dt.float32, tag="convps", name="convps")
                for cb in range(2):
                    for k in range(3):
                        rhs = t_xP[cb][:, c * CH + HALO - 1 + k:
                                       c * CH + HALO - 1 + k + CH]
                        nc.tensor.matmul(ps[0:44, :], t_wconv[cb][k][:], rhs,
                                         start=(cb == 0 and k == 0),
                                         stop=(cb == 1 and k == 2))
                t, rb = c // 4, 32 * (c % 4)
                nc.scalar.activation(pk[t][rb:rb + 12, :], ps[0:12, :], AF.Identity,
                                     bias=t_boff[:], scale=1.0)
                nc.scalar.activation(mk[t][rb:rb + 12, :], ps[32:44, :], AF.Sigmoid,
                                     bias=t_bmask[:], scale=1.0)

            # ---- small chain (packed [96, CH]) ----------------------------
            for t in range(2):
                nc.vector.tensor_add(pk[t][:], pk[t][:], t_iot[t][:])
                nc.vector.tensor_copy(i16p[t][:], pk[t][:])         # ~round/trunc
                nc.vector.tensor_copy(p0f[t][:], i16p[t][:])
                nc.vector.tensor_sub(ttl[t][:], pk[t][:], p0f[t][:])  # d = p - i
                # floor fixup: if d < 0: i -= 1, d += 1
                nc.vector.tensor_scalar(pk[t][:], ttl[t][:], 0.0, None, ALU.is_lt)
                nc.vector.tensor_sub(p0f[t][:], p0f[t][:], pk[t][:])
                nc.vector.tensor_add(ttl[t][:], ttl[t][:], pk[t][:])  # t in [0,1)
                nc.vector.tensor_mul(ttl[t][:], ttl[t][:], mk[t][:])
                nc.vector.tensor_copy(a1p[t][:], ttl[t][:])
                nc.vector.tensor_sub(a0p[t][:], mk[t][:], ttl[t][:])
                nc.vector.tensor_copy(i16p[t][:], p0f[t][:])
                nc.vector.tensor_scalar_max(i16p[t][:], i16p[t][:], 0)
                nc.vector.tensor_scalar_min(i16p[t][:], i16p[t][:], W - 1)

            # ---- unpack to planes -----------------------------------------
            for c in range(NCHUNK):
                t, rb = c // 4, 32 * (c % 4)
                nc.sync.dma_start(aplane[0:12, c * CH:(c + 1) * CH],
                                  a0p[t][rb:rb + 12, :])
                nc.sync.dma_start(aplane[12:24, c * CH:(c + 1) * CH],
                                  a1p[t][rb:rb + 12, :])

            # ---- idx spread (call-major sigma):
            # idxt[dk][16g+p, 32c+s] = i16p[12*(p//2)+dk, (p%2)*256+32c+s]
            #   => gather call c, output col j=s*16+p holds plane position
            #      l'(j) = 256*(j%16) + 32*c + j//16
            for dk in range(12):
                for g in range(8):
                    nc.sync.dma_start(idxt[dk][16 * g:16 * g + 8, :],
                                      i16p[0][dk:128:32, :])
                    nc.sync.dma_start(idxt[dk][16 * g + 8:16 * g + 16, :],
                                      i16p[1][dk:128:32, :])
            # ---- gather / broadcast / modulate (8 calls of 512) -----------
            shalf = [spool.tile([128, LH], dt.bfloat16, tag=f"s{kb}", name=f"s{kb}")
                     for kb in range(6)]
            ap_sig = aplane[:].rearrange("a (p c s) -> a c s p", p=16, c=8, s=32)
            for c in range(8):
                for dk in range(12):
                    gt = gpool.tile([128, CH], dt.bfloat16, tag=f"g{dk}", name=f"g{dk}")
                    at = apool.tile([128, CH], dt.bfloat16, tag=f"a{dk}", name=f"a{dk}")
                    nc.gpsimd.dma_gather(
                        gt[:].unsqueeze(1),
                        t_xT[dk // 3][:], idxt[dk][:, 32 * c:32 * c + 32],
                        num_idxs=CH, num_idxs_reg=CH, elem_size=128,
                        transpose=True, queue_num=dk % 2,
                        sbuf_tokens_per_rank=128,
                        sbuf_free_dim_per_rank=256)
                    bps = psb.tile([128, CH], dt.float32, tag="bcps", name="bcps")
                    nc.tensor.matmul(bps[:], t_wsel[dk][:], ap_sig[:, c],
                                     start=True, stop=True)
                    nc.scalar.copy(at[:], bps[:])
                    mt = mpool.tile([128, CH], dt.bfloat16, tag=f"mt{dk % 2}", name=f"mt{dk % 2}")
                    nc.vector.tensor_mul(mt[:], gt[:], at[:])
                    v1t = mpool.tile([64, CH], dt.bfloat16, tag=f"v1t{dk % 2}", name=f"v1t{dk % 2}")
                    nc.sync.dma_start(v1t[:], mt[64:128, :])
                    nc.vector.tensor_add(
                        shalf[dk // 2][64 * (dk % 2):64 * (dk % 2) + 64,
                                       c * CH:(c + 1) * CH],
                        mt[0:64, :], v1t[:])
            # ---- main matmuls ---------------------------------------------
            for cn in range(NCHUNK):
                for mb in range(2):
                    mps = psm.tile([128, CH], dt.float32, tag=f"mps{mb}", name=f"mps{mb}")
                    for kb in range(6):
                        rhs = shalf[kb][:].rearrange(
                            "a (c s p) -> a p c s", c=8, s=32, p=16)[:, 2 * cn:2 * cn + 2]
                        nc.tensor.matmul(mps[:], t_wmain[kb][mb][:], rhs,
                                         start=(kb == 0), stop=(kb == 5))
                    ot = opool.tile([128, CH], dt.float32, tag=f"ot{mb}", name=f"ot{mb}")
                    nc.scalar.activation(ot[:], mps[:], AF.Identity,
                                         bias=t_bmain[mb][:], scale=1.0)
                    nc.sync.dma_start(yout[mb, :, cn * CH:(cn + 1) * CH], ot[:])

    nc.compile()
    return nc


# ---------------------------------------------------------------------------

def _prep_core_inputs(x, w_off, b_off, w_mask, b_mask, weight, bias, b, h):
    q0 = h * LH - HALO
    xpad = np.zeros((C, W + 1), np.float32)
    lo, hi = max(0, q0), min(L, q0 + W + 1)
    xpad[:, lo - q0:hi - q0] = x[b][:, lo:hi]
    xpad_bf = xpad.astype(bf16)

    xT = np.zeros((4, 128, WROWS), bf16)
    for d in range(4):
        rows = np.concatenate([xpad_bf[d * 64:(d + 1) * 64, :W],
                               xpad_bf[d * 64:(d + 1) * 64, 1:W + 1]],
                              axis=0).T           # [W, 128] row e
        full = np.zeros((WROWS, 128), bf16)
        full[:W] = rows
        xT[d] = full.reshape(WROWS // 128, 128, 128).transpose(1, 0, 2) \
                    .reshape(128, WROWS)
    xP = np.ascontiguousarray(xpad_bf[:, :W].reshape(2, 128, W))

    wconv = np.zeros((2, 3, 128, 44), bf16)
    for cb in range(2):
        for k in range(3):
            wconv[cb, k, :, 0:12] = w_off[:, cb * 128:(cb + 1) * 128, k].T
            wconv[cb, k, :, 32:44] = w_mask[:, cb * 128:(cb + 1) * 128, k].T
    iotas = np.zeros((2, 128, CH), np.float32)
    col = np.arange(CH, dtype=np.float32)
    for t in range(2):
        for cb in range(4):
            for r in range(12):
                iotas[t, 32 * cb + r, :] = 512 * (4 * t + cb) + col + (r % 3) - 1 + HALO
    boff_c = b_off.astype(np.float32).reshape(12, 1)
    bmask_c = b_mask.astype(np.float32).reshape(12, 1)

    wmain = np.zeros((6, 2, 128, 128), bf16)
    for kb in range(6):
        for half in range(2):
            dk = 2 * kb + half
            d, k = dk // 3, dk % 3
            wblock = weight[:, d * 64:(d + 1) * 64, k]
            for mb in range(2):
                wmain[kb, mb, 64 * half:64 * half + 64, :] = \
                    wblock[mb * 128:(mb + 1) * 128, :].T
    bmain = bias.astype(np.float32).reshape(2, 128, 1)

    wsel = np.zeros((12, 24, 128), bf16)
    for dk in range(12):
        wsel[dk, dk, 0:64] = 1.0
        wsel[dk, 12 + dk, 64:128] = 1.0
    return {"xT": xT, "xP": xP, "wconv": wconv, "iotas": iotas,
            "boff": boff_c, "bmask": bmask_c,
            "wmain": wmain, "bmain": bmain, "wsel": wsel}


_CACHED = {}


def kernel(x, w_off, b_off, w_mask, b_mask, weight, bias):
    x = np.asarray(x, np.float32)
    w_off = np.asarray(w_off, np.float32)
    b_off = np.asarray(b_off, np.float32)
    w_mask = np.asarray(w_mask, np.float32)
    b_mask = np.asarray(b_mask, np.float32)
    weight = np.asarray(weight, np.float32)
    bias = np.asarray(bias, np.float32)

    if "nc" not in _CACHED:
        _CACHED["nc"] = build_program(1)
    nc = _CACHED["nc"]

    in_maps = [
        _prep_core_inputs(x, w_off, b_off, w_mask, b_mask, weight, bias,
                          core // 2, core % 2)
        for core in range(N_CORES)
    ]
    from concourse.bass_utils import run_bass_kernel_spmd
    res = run_bass_kernel_spmd(nc, in_maps, core_ids=list(range(N_CORES)))
    out = np.zeros((B, C, L), np.float32)
    for core in range(N_CORES):
        b, h = core // 2, core % 2
        y = res.results[core]["y"]
        out[b, 0:128, h * LH:(h + 1) * LH] = y[0]
        out[b, 128:256, h * LH:(h + 1) * LH] = y[1]
    return out



# revision 5
# speedup vs baseline: 1.2042x; 1.2042x over previous
"""Deformable Conv1d (B=4, C=256, L=8192, K=3, DG=4) on 8 Trainium2 cores.

Sharding: core = (sample b = core//2, L-half h = core%2); each core computes
out[b, :, h*4096:(h+1)*4096] from a haloed window of x[b].

Per-core pipeline (2 waves of 2048 output positions):
  conv (PE, bf16): offset/mask convs as shifted-rhs matmuls; Act drains
    PSUM with per-row bias; DVE adds the iota plane so pk holds window
    positions p.
  chain (DVE/Act, [128,512] packed): floor via round+fixup, a0=(1-t)m,
    a1=t*m, idx=int16 clamp.
  idx spread: 12+7 flat-order DMAs build the 16-wrapped, 8-replicated
    gather index plane per wave.
  dma_gather (transpose, SBUF pair tables): 12 gathers of 2048 idx/wave;
    gather col j holds plane position l_w = 128*(j%16) + j//16.
  selector matmul (PE) broadcasts a0/a1 to the 128-partition pair layout
    through a sigma-permuted aplane AP; Act drains PSUM->bf16.
  modulate (DVE): two same-base-partition muls + pair add into S.
  main matmul (PE): accumulates W_kb @ S_kb progressively as waves land;
    cn' in {0,1} during the wave, {2,3} chased at the next wave's start.
"""
import sys
sys.path.insert(0, '/opt/trn_rl_repo')
from contextlib import ExitStack
import numpy as np
import ml_dtypes

import concourse.bass as bass
import concourse.tile as tile
from concourse import bacc, mybir

dt = mybir.dt
bf16 = ml_dtypes.bfloat16

B, C, L = 4, 256, 8192
N_CORES = 8
LH = L // 2
HALO = 17
W = LH + 2 * HALO          # 4130 window positions
WROWS = 33 * 128           # 4224 padded rows in pair tables
WAVE = 2048
AF = mybir.ActivationFunctionType
ALU = mybir.AluOpType


def build_program(n_reps=1):
    nc = bacc.Bacc("TRN2", target_bir_lowering=False, debug=False,
                   enable_asserts=True, num_devices=N_CORES,
                   num_swdge_queues=2, dynamic_dma_scratch_size=24576)

    def din(name, shape, dty):
        return nc.dram_tensor(name, shape, dty, kind="ExternalInput").ap()

    xT = din("xT", (4, 128, WROWS), dt.bfloat16)
    xP = din("xP", (2, 128, W), dt.bfloat16)
    wconv = din("wconv", (2, 3, 128, 44), dt.bfloat16)
    iotas = din("iotas", (2, 128, 512), dt.float32)
    boff = din("boff", (12, 1), dt.float32)
    bmask = din("bmask", (12, 1), dt.float32)
    wmain = din("wmain", (6, 2, 128, 128), dt.bfloat16)
    bmain = din("bmain", (2, 128, 1), dt.float32)
    wsel = din("wsel", (12, 24, 128), dt.bfloat16)
    yout = nc.dram_tensor("y", (2, 128, LH), dt.float32, kind="ExternalOutput").ap()

    with ExitStack() as ctx:
        tc = ctx.enter_context(tile.TileContext(nc))
        cpool = ctx.enter_context(tc.tile_pool(name="const", bufs=1))
        chpool = ctx.enter_context(tc.tile_pool(name="chain", bufs=2))
        splane = ctx.enter_context(tc.tile_pool(name="spl", bufs=1))
        gpool = ctx.enter_context(tc.tile_pool(name="g", bufs=2))
        apool = ctx.enter_context(tc.tile_pool(name="a", bufs=2))
        mpool = ctx.enter_context(tc.tile_pool(name="mtp", bufs=1))
        opool = ctx.enter_context(tc.tile_pool(name="o", bufs=3))
        psc = ctx.enter_context(tc.tile_pool(name="psc", bufs=2, space="PSUM"))
        psb = ctx.enter_context(tc.tile_pool(name="psb", bufs=2, space="PSUM"))
        psm = ctx.enter_context(tc.tile_pool(name="psm", bufs=1, space="PSUM"))

        t_xT = [cpool.tile([128, WROWS], dt.bfloat16, tag=f"xT{d}", name=f"xT{d}")
                for d in range(4)]
        for d in range(4):
            nc.sync.dma_start(t_xT[d][:], xT[d])
        t_xP = [cpool.tile([128, W], dt.bfloat16, tag=f"xP{cb}", name=f"xP{cb}")
                for cb in range(2)]
        for cb in range(2):
            nc.sync.dma_start(t_xP[cb][:], xP[cb])
        t_wconv = [[cpool.tile([128, 44], dt.bfloat16, tag=f"wc{cb}{k}",
                               name=f"wc{cb}{k}") for k in range(3)]
                   for cb in range(2)]
        for cb in range(2):
            for k in range(3):
                nc.sync.dma_start(t_wconv[cb][k][:], wconv[cb, k])
        t_iot = [cpool.tile([128, 512], dt.float32, tag=f"iot{t}", name=f"iot{t}")
                 for t in range(2)]
        for t in range(2):
            nc.sync.dma_start(t_iot[t][:], iotas[t])
        t_boff = cpool.tile([12, 1], dt.float32, name="boff")
        nc.sync.dma_start(t_boff[:], boff[:])
        t_bmask = cpool.tile([12, 1], dt.float32, name="bmask")
        nc.sync.dma_start(t_bmask[:], bmask[:])
        t_wmain = [[cpool.tile([128, 128], dt.bfloat16, tag=f"wm{kb}{mb}",
                               name=f"wm{kb}{mb}") for mb in range(2)]
                   for kb in range(6)]
        for kb in range(6):
            for mb in range(2):
                nc.sync.dma_start(t_wmain[kb][mb][:], wmain[kb, mb])
        t_bmain = [cpool.tile([128, 1], dt.float32, tag=f"bm{mb}", name=f"bm{mb}")
                   for mb in range(2)]
        for mb in range(2):
            nc.sync.dma_start(t_bmain[mb][:], bmain[mb])
        t_wsel = [cpool.tile([24, 128], dt.bfloat16, tag=f"sel{dk}",
                             name=f"sel{dk}") for dk in range(12)]
        for dk in range(12):
            nc.sync.dma_start(t_wsel[dk][:], wsel[dk])

        for _rep in range(n_reps):
            aplane = splane.tile([24, LH], dt.bfloat16, tag="aplane", name="aplane")
            # sigma AP: col = 2048*q + 128*p + 32*u + h
            ap_sig = aplane[:].rearrange("a (q p u h) -> a q u h p",
                                         q=2, p=16, u=4, h=32)
            shalf = [splane.tile([128, LH], dt.bfloat16, tag=f"s{kb}",
                                 name=f"s{kb}") for kb in range(6)]
            # main rhs AP: col = 2048*q + 16*r + g, slice g in [4c',4c'+4)
            sh_sig = [shalf[kb][:].rearrange("a (q r g) -> a q g r",
                                             q=2, r=128, g=16) for kb in range(6)]
            idxw = [splane.tile([128, 1536], dt.int16, tag=f"idx{w}",
                                name=f"idx{w}") for w in range(2)]

            def main_mm(w, cps):
                tiles = {}
                for cp in cps:
                    for mb in range(2):
                        tiles[(cp, mb)] = psm.tile(
                            [128, 512], dt.float32, tag=f"mps{cp % 2}{mb}",
                            name="mps")
                return tiles

            def main_step(tiles, w, kb, cps):
                for cp in cps:
                    for mb in range(2):
                        mps = tiles[(cp, mb)]
                        nc.tensor.matmul(mps[:], t_wmain[kb][mb][:],
                                         sh_sig[kb][:, w, 4 * cp:4 * cp + 4, :],
                                         start=(kb == 0), stop=(kb == 5))
                        if kb == 5:
                            cn = 4 * w + cp
                            ot = opool.tile([128, 512], dt.float32, tag="ot",
                                            name="ot")
                            nc.scalar.activation(ot[:], mps[:], AF.Identity,
                                                 bias=t_bmain[mb][:], scale=1.0)
                            nc.sync.dma_start(
                                yout[mb, :, cn * 512:(cn + 1) * 512], ot[:])

            def chase(w):
                tiles = main_mm(w, (2, 3))
                for kb in range(6):
                    main_step(tiles, w, kb, (2, 3))

            for w in range(2):
                # ---- conv + drains ---------------------------------------
                pk = chpool.tile([128, 512], dt.float32, tag="pk", name="pk")
                mk = chpool.tile([128, 512], dt.float32, tag="mk", name="mk")
                for cb in range(4):
                    c = 4 * w + cb
                    ps = psc.tile([44, 512], dt.float32, tag="convps", name="convps")
                    for xb in range(2):
                        for k in range(3):
                            rhs = t_xP[xb][:, c * 512 + HALO - 1 + k:
                                           c * 512 + HALO - 1 + k + 512]
                            nc.tensor.matmul(ps[:], t_wconv[xb][k][:], rhs,
                                             start=(xb == 0 and k == 0),
                                             stop=(xb == 1 and k == 2))
                    rb = 32 * cb
                    nc.scalar.activation(pk[rb:rb + 12, :], ps[0:12, :],
                                         AF.Identity, bias=t_boff[:], scale=1.0)
                    nc.scalar.activation(mk[rb:rb + 12, :], ps[32:44, :],
                                         AF.Sigmoid, bias=t_bmask[:], scale=1.0)

                # ---- small chain -----------------------------------------
                i16r = chpool.tile([128, 512], dt.int16, tag="i16r", name="i16r",
                                   bufs=1)
                p0f = chpool.tile([128, 512], dt.float32, tag="p0f", name="p0f",
                                  bufs=1)
                ttl = chpool.tile([128, 512], dt.float32, tag="ttl", name="ttl",
                                  bufs=1)
                msk = chpool.tile([128, 512], dt.float32, tag="msk", name="msk",
                                  bufs=1)
                a0p = chpool.tile([128, 512], dt.bfloat16, tag="a0p", name="a0p")
                a1p = chpool.tile([128, 512], dt.bfloat16, tag="a1p", name="a1p")
                i16p = chpool.tile([128, 512], dt.int16, tag="i16p", name="i16p")
                nc.vector.tensor_add(pk[:], pk[:], t_iot[w][:])
                nc.scalar.copy(i16r[:], pk[:])            # round to nearest
                nc.scalar.copy(p0f[:], i16r[:])
                nc.vector.tensor_sub(ttl[:], pk[:], p0f[:])
                nc.vector.tensor_scalar(msk[:], ttl[:], 0.0, None, ALU.is_lt)
                nc.vector.tensor_sub(p0f[:], p0f[:], msk[:])
                nc.vector.tensor_add(ttl[:], ttl[:], msk[:])  # t in [0,1)
                nc.vector.tensor_mul(ttl[:], ttl[:], mk[:])   # t*m
                nc.scalar.copy(a1p[:], ttl[:])
                nc.vector.tensor_sub(a0p[:], mk[:], ttl[:])   # m - t*m
                nc.vector.tensor_scalar(i16p[:], p0f[:], 0.0, float(W - 1),
                                        ALU.max, ALU.min)

                # ---- unpack a-planes + idx spread ------------------------
                for cb in range(4):
                    c = 4 * w + cb
                    nc.scalar.dma_start(aplane[0:12, c * 512:(c + 1) * 512],
                                        a0p[32 * cb:32 * cb + 12, :])
                    nc.scalar.dma_start(aplane[12:24, c * 512:(c + 1) * 512],
                                        a1p[32 * cb:32 * cb + 12, :])
                for dk in range(12):
                    nc.sync.dma_start(idxw[w][0:16, dk * 128:(dk + 1) * 128],
                                      i16p[dk:128:32, :])
                for q in range(1, 8):
                    nc.sync.dma_start(idxw[w][16 * q:16 * q + 16, :],
                                      idxw[w][0:16, :])

                if w == 1:
                    chase(0)
                mtiles = main_mm(w, (0, 1))

                # ---- gather / broadcast / modulate / main ----------------
                for dk in range(12):
                    kb, h = dk // 2, dk % 2
                    gt = gpool.tile([128, WAVE], dt.bfloat16, tag="gt", name="gt")
                    for u in range(4):
                        nc.gpsimd.dma_gather(
                            gt[:, 512 * u:512 * (u + 1)].unsqueeze(1),
                            t_xT[dk // 3][:],
                            idxw[w][:, dk * 128 + 32 * u:dk * 128 + 32 * u + 32],
                            num_idxs=512, num_idxs_reg=512, elem_size=128,
                            transpose=True, queue_num=u % 2,
                            sbuf_tokens_per_rank=128,
                            sbuf_free_dim_per_rank=256)
                    at = apool.tile([128, WAVE], dt.bfloat16, tag="at", name="at")
                    for u in range(4):
                        bps = psb.tile([128, 512], dt.float32, tag="bcps",
                                       name="bcps")
                        nc.tensor.matmul(bps[:], t_wsel[dk][:], ap_sig[:, w, u],
                                         start=True, stop=True)
                        nc.scalar.copy(at[:, u * 512:(u + 1) * 512], bps[:])
                    mt0 = mpool.tile([64, WAVE], dt.bfloat16, tag="mt0",
                                     name="mt0")
                    mt1 = mpool.tile([64, WAVE], dt.bfloat16, tag="mt1",
                                     name="mt1")
                    nc.vector.tensor_mul(mt0[:], gt[0:64, :], at[0:64, :])
                    nc.vector.tensor_mul(mt1[:], gt[64:128, :], at[64:128, :])
                    nc.vector.tensor_add(
                        shalf[kb][64 * h:64 * h + 64, w * WAVE:(w + 1) * WAVE],
                        mt0[:], mt1[:])
                    if h == 1:
                        main_step(mtiles, w, kb, (0, 1))
            chase(1)

    nc.compile()
    return nc


# ---------------------------------------------------------------------------

def _prep_core_inputs(x, w_off, b_off, w_mask, b_mask, weight, bias, b, h):
    q0 = h * LH - HALO
    xpad = np.zeros((C, W + 1), np.float32)
    lo, hi = max(0, q0), min(L, q0 + W + 1)
    xpad[:, lo - q0:hi - q0] = x[b][:, lo:hi]
    xpad_bf = xpad.astype(bf16)

    xT = np.zeros((4, 128, WROWS), bf16)
    for d in range(4):
        rows = np.concatenate([xpad_bf[d * 64:(d + 1) * 64, :W],
                               xpad_bf[d * 64:(d + 1) * 64, 1:W + 1]],
                              axis=0).T           # [W, 128] row e
        full = np.zeros((WROWS, 128), bf16)
        full[:W] = rows
        xT[d] = full.reshape(WROWS // 128, 128, 128).transpose(1, 0, 2) \
                    .reshape(128, WROWS)
    xP = np.ascontiguousarray(xpad_bf[:, :W].reshape(2, 128, W))

    wconv = np.zeros((2, 3, 128, 44), bf16)
    for cb in range(2):
        for k in range(3):
            wconv[cb, k, :, 0:12] = w_off[:, cb * 128:(cb + 1) * 128, k].T
            wconv[cb, k, :, 32:44] = w_mask[:, cb * 128:(cb + 1) * 128, k].T
    iotas = np.zeros((2, 128, 512), np.float32)
    col = np.arange(512, dtype=np.float32)
    for t in range(2):
        for cb in range(4):
            for r in range(12):
                iotas[t, 32 * cb + r, :] = \
                    512 * (4 * t + cb) + col + (r % 3) - 1 + HALO
    boff_c = b_off.astype(np.float32).reshape(12, 1)
    bmask_c = b_mask.astype(np.float32).reshape(12, 1)

    wmain = np.zeros((6, 2, 128, 128), bf16)
    for kb in range(6):
        for half in range(2):
            dk = 2 * kb + half
            d, k = dk // 3, dk % 3
            wblock = weight[:, d * 64:(d + 1) * 64, k]
            for mb in range(2):
                wmain[kb, mb, 64 * half:64 * half + 64, :] = \
                    wblock[mb * 128:(mb + 1) * 128, :].T
    bmain = bias.astype(np.float32).reshape(2, 128, 1)

    wsel = np.zeros((12, 24, 128), bf16)
    for dk in range(12):
        wsel[dk, dk, 0:64] = 1.0
        wsel[dk, 12 + dk, 64:128] = 1.0
    return {"xT": xT, "xP": xP, "wconv": wconv, "iotas": iotas,
            "boff": boff_c, "bmask": bmask_c,
            "wmain": wmain, "bmain": bmain, "wsel": wsel}


_CACHED = {}


def kernel(x, w_off, b_off, w_mask, b_mask, weight, bias):
    x = np.asarray(x, np.float32)
    w_off = np.asarray(w_off, np.float32)
    b_off = np.asarray(b_off, np.float32)
    w_mask = np.asarray(w_mask, np.float32)
    b_mask = np.asarray(b_mask, np.float32)
    weight = np.asarray(weight, np.float32)
    bias = np.asarray(bias, np.float32)

    if "nc" not in _CACHED:
        _CACHED["nc"] = build_program(1)
    nc = _CACHED["nc"]

    in_maps = [
        _prep_core_inputs(x, w_off, b_off, w_mask, b_mask, weight, bias,
                          core // 2, core % 2)
        for core in range(N_CORES)
    ]
    from concourse.bass_utils import run_bass_kernel_spmd
    res = run_bass_kernel_spmd(nc, in_maps, core_ids=list(range(N_CORES)))
    out = np.zeros((B, C, L), np.float32)
    for core in range(N_CORES):
        b, h = core // 2, core % 2
        y = res.results[core]["y"]
        out[b, 0:128, h * LH:(h + 1) * LH] = y[0]
        out[b, 128:256, h * LH:(h + 1) * LH] = y[1]
    return out


# revision 11
# speedup vs baseline: 1.2410x; 1.0306x over previous
"""Deformable Conv1d (B=4, C=256, L=8192, K=3, DG=4) on 8 Trainium2 cores.

Sharding: core = (sample b = core//2, L-half h = core%2); each core computes
out[b, :, h*4096:(h+1)*4096] from a haloed window of x[b].

Per-core pipeline (2 waves of 2048 output positions):
  conv (PE, bf16): offset/mask convs as shifted-rhs matmuls; Act drains
    PSUM with per-row bias; DVE adds the iota plane so pk holds window
    positions p.
  chain (DVE/Act, [128,512] packed): floor via round+fixup, a0=(1-t)m,
    a1=t*m, idx=int16 clamp.
  idx spread: 12+7 flat-order DMAs build the 16-wrapped, 8-replicated
    gather index plane per wave.
  dma_gather (transpose, SBUF pair tables): 12 gathers of 2048 idx/wave;
    gather col j holds plane position l_w = 128*(j%16) + j//16.
  selector matmul (PE) broadcasts a0/a1 to the 128-partition pair layout
    through a sigma-permuted aplane AP; Act drains PSUM->bf16.
  modulate (DVE): two same-base-partition muls + pair add into S.
  main matmul (PE): accumulates W_kb @ S_kb progressively as waves land;
    cn' in {0,1} during the wave, {2,3} chased at the next wave's start.
"""
import os
# Subtile dependency tracking misses deps for strided-partition DMA APs
# (e.g. reading i16p[dk:128:32]); force whole-tensor deps.
os.environ.setdefault("BY_DEFAULT_DISABLE_SUBTILE_DEPS", "1")
import sys
sys.path.insert(0, '/opt/trn_rl_repo')
from contextlib import ExitStack
import numpy as np
import ml_dtypes

import concourse.bass as bass
import concourse.tile as tile
from concourse import bacc, mybir

dt = mybir.dt
bf16 = ml_dtypes.bfloat16

B, C, L = 4, 256, 8192
N_CORES = 8
LH = L // 2
HALO = 17
W = LH + 2 * HALO          # 4130 window positions
WROWS = 33 * 128           # 4224 padded rows in pair tables
WAVE = 2048
AF = mybir.ActivationFunctionType
ALU = mybir.AluOpType


def build_program(n_reps=1):
    nc = bacc.Bacc("TRN2", target_bir_lowering=False, debug=False,
                   enable_asserts=True, num_devices=N_CORES,
                   num_swdge_queues=4, dynamic_dma_scratch_size=24576)

    def din(name, shape, dty):
        return nc.dram_tensor(name, shape, dty, kind="ExternalInput").ap()

    xT = din("xT", (4, 128, WROWS), dt.bfloat16)
    xP = din("xP", (2, 128, W), dt.bfloat16)
    wconv = din("wconv", (2, 3, 128, 64), dt.bfloat16)
    iotas = din("iotas", (2, 128, 512), dt.float32)
    boff = din("boff", (32, 1), dt.float32)
    bmask = din("bmask", (32, 1), dt.float32)
    wmain = din("wmain", (6, 2, 128, 128), dt.bfloat16)
    bmain = din("bmain", (2, 128, 1), dt.float32)
    wsel = din("wsel", (12, 24, 128), dt.bfloat16)
    yout = nc.dram_tensor("y", (2, 128, LH), dt.float32, kind="ExternalOutput").ap()

    with ExitStack() as ctx:
        tc = ctx.enter_context(tile.TileContext(nc))
        cpool = ctx.enter_context(tc.tile_pool(name="const", bufs=1))
        chpool = ctx.enter_context(tc.tile_pool(name="chain", bufs=2))
        splane = ctx.enter_context(tc.tile_pool(name="spl", bufs=1))
        gpool = ctx.enter_context(tc.tile_pool(name="g", bufs=3))
        apool = ctx.enter_context(tc.tile_pool(name="a", bufs=2))
        mpool = ctx.enter_context(tc.tile_pool(name="mtp", bufs=1))
        opool = ctx.enter_context(tc.tile_pool(name="o", bufs=3))
        aux = ctx.enter_context(tc.tile_pool(name="aux", bufs=2, space="PSUM"))
        psb = ctx.enter_context(tc.tile_pool(name="psb", bufs=2, space="PSUM"))
        psm = ctx.enter_context(tc.tile_pool(name="psm", bufs=1, space="PSUM"))

        t_xT = [cpool.tile([128, WROWS], dt.bfloat16, tag=f"xT{d}", name=f"xT{d}")
                for d in range(4)]
        for d in range(4):
            nc.sync.dma_start(t_xT[d][:], xT[d])
        t_xP = [cpool.tile([128, W], dt.bfloat16, tag=f"xP{cb}", name=f"xP{cb}")
                for cb in range(2)]
        for cb in range(2):
            nc.sync.dma_start(t_xP[cb][:], xP[cb])
        t_wconv = [[cpool.tile([128, 64], dt.bfloat16, tag=f"wc{cb}{k}",
                               name=f"wc{cb}{k}") for k in range(3)]
                   for cb in range(2)]
        for cb in range(2):
            for k in range(3):
                nc.sync.dma_start(t_wconv[cb][k][:], wconv[cb, k])
        t_iot = [cpool.tile([128, 512], dt.float32, tag=f"iot{t}", name=f"iot{t}")
                 for t in range(2)]
        for t in range(2):
            nc.sync.dma_start(t_iot[t][:], iotas[t])
        t_boff = cpool.tile([32, 1], dt.float32, name="boff")
        nc.sync.dma_start(t_boff[:], boff[:])
        t_bmask = cpool.tile([32, 1], dt.float32, name="bmask")
        nc.sync.dma_start(t_bmask[:], bmask[:])
        t_wmain = [[cpool.tile([128, 128], dt.bfloat16, tag=f"wm{kb}{mb}",
                               name=f"wm{kb}{mb}") for mb in range(2)]
                   for kb in range(6)]
        for kb in range(6):
            for mb in range(2):
                nc.sync.dma_start(t_wmain[kb][mb][:], wmain[kb, mb])
        t_bmain = [cpool.tile([128, 1], dt.float32, tag=f"bm{mb}", name=f"bm{mb}")
                   for mb in range(2)]
        for mb in range(2):
            nc.sync.dma_start(t_bmain[mb][:], bmain[mb])
        t_wsel = [cpool.tile([24, 128], dt.bfloat16, tag=f"sel{dk}",
                             name=f"sel{dk}") for dk in range(12)]
        for dk in range(12):
            nc.sync.dma_start(t_wsel[dk][:], wsel[dk])

        for _rep in range(n_reps):
            aplane = splane.tile([24, LH], dt.bfloat16, tag="aplane", name="aplane")
            # sigma AP: col = 2048*q + 128*p + 32*u + h
            ap_sig = aplane[:].rearrange("a (q p u h) -> a q u h p",
                                         q=2, p=16, u=4, h=32)
            shalf = [splane.tile([128, LH], dt.bfloat16, tag=f"s{kb}",
                                 name=f"s{kb}") for kb in range(6)]
            # main rhs AP: col = 2048*q + 16*r + g, slice g in [4c',4c'+4)
            sh_sig = [shalf[kb][:].rearrange("a (q r g) -> a q g r",
                                             q=2, r=128, g=16) for kb in range(6)]
            idxw = [splane.tile([128, 1536], dt.int16, tag=f"idx{w}",
                                name=f"idx{w}") for w in range(2)]

            def mm_step(tiles, w, kb, cps, pfx=""):
                for cp in cps:
                    for mb in range(2):
                        mps = tiles[(cp, mb)]
                        nc.tensor.matmul(mps[:], t_wmain[kb][mb][:],
                                         sh_sig[kb][:, w, 4 * cp:4 * cp + 4, :],
                                         start=(kb == 0), stop=(kb == 5))
                        if kb == 5:
                            cn = 4 * w + cp
                            ot = opool.tile([128, 512], dt.float32, tag="ot",
                                            name="ot")
                            nc.scalar.activation(ot[:], mps[:], AF.Identity,
                                                 bias=t_bmain[mb][:], scale=1.0)
                            nc.sync.dma_start(
                                yout[mb, :, cn * 512:(cn + 1) * 512], ot[:])

            def chase_tiles():
                return {(cp, mb): None for cp in (2, 3) for mb in range(2)}

            def chase_phase(w, cp, kb, tiles):
                # aux-pool accumulators, one cp at a time (2 banks)
                if kb == 0:
                    for mb in range(2):
                        tiles[(cp, mb)] = aux.tile([128, 512], dt.float32,
                                                   tag="aux", name="cps")
                mm_step(tiles, w, kb, (cp,))

            for w in range(2):
                # ---- conv + drains ---------------------------------------
                pk = chpool.tile([128, 512], dt.float32, tag="pk", name="pk")
                mk = chpool.tile([128, 512], dt.float32, tag="mk", name="mk")
                for cb in range(4):
                    c = 4 * w + cb
                    ps = aux.tile([64, 512], dt.float32, tag="aux", name="convps")
                    for xb in range(2):
                        for k in range(3):
                            rhs = t_xP[xb][:, c * 512 + HALO - 1 + k:
                                           c * 512 + HALO - 1 + k + 512]
                            nc.tensor.matmul(ps[:], t_wconv[xb][k][:], rhs,
                                             start=(xb == 0 and k == 0),
                                             stop=(xb == 1 and k == 2))
                    rb = 32 * cb
                    nc.scalar.activation(pk[rb:rb + 32, :], ps[0:32, :],
                                         AF.Identity, bias=t_boff[:], scale=1.0)
                    nc.scalar.activation(mk[rb:rb + 32, :], ps[32:64, :],
                                         AF.Sigmoid, bias=t_bmask[:], scale=1.0)

                # ---- small chain -----------------------------------------
                i16r = chpool.tile([128, 512], dt.int16, tag="i16r", name="i16r",
                                   bufs=1)
                p0f = chpool.tile([128, 512], dt.float32, tag="p0f", name="p0f",
                                  bufs=1)
                ttl = chpool.tile([128, 512], dt.float32, tag="ttl", name="ttl",
                                  bufs=1)
                msk = chpool.tile([128, 512], dt.float32, tag="msk", name="msk",
                                  bufs=1)
                a0p = chpool.tile([128, 512], dt.bfloat16, tag="a0p", name="a0p")
                a1p = chpool.tile([128, 512], dt.bfloat16, tag="a1p", name="a1p")
                i16p = chpool.tile([128, 512], dt.int16, tag="i16p", name="i16p")
                nc.vector.tensor_add(pk[:], pk[:], t_iot[w][:])
                nc.scalar.copy(i16r[:], pk[:])            # round to nearest
                nc.scalar.copy(p0f[:], i16r[:])
                nc.vector.tensor_sub(ttl[:], pk[:], p0f[:])
                nc.vector.tensor_scalar(msk[:], ttl[:], 0.0, None, ALU.is_lt)
                nc.vector.tensor_sub(p0f[:], p0f[:], msk[:])
                nc.vector.tensor_add(ttl[:], ttl[:], msk[:])  # t in [0,1)
                nc.vector.tensor_mul(ttl[:], ttl[:], mk[:])   # t*m
                nc.scalar.copy(a1p[:], ttl[:])
                nc.vector.tensor_sub(a0p[:], mk[:], ttl[:])   # m - t*m
                nc.vector.tensor_scalar(i16p[:], p0f[:], 0.0, float(W - 1),
                                        ALU.max, ALU.min)

                # ---- unpack a-planes + idx spread ------------------------
                for cb in range(4):
                    c = 4 * w + cb
                    nc.scalar.dma_start(aplane[0:12, c * 512:(c + 1) * 512],
                                        a0p[32 * cb:32 * cb + 12, :])
                    nc.scalar.dma_start(aplane[12:24, c * 512:(c + 1) * 512],
                                        a1p[32 * cb:32 * cb + 12, :])
                for dk in range(12):
                    nc.sync.dma_start(idxw[w][0:16, dk * 128:(dk + 1) * 128],
                                      i16p[dk:128:32, :])
                for q in range(1, 8):
                    nc.sync.dma_start(idxw[w][16 * q:16 * q + 16, :],
                                      idxw[w][0:16, :])

                mtiles = {(cp, mb): psm.tile([128, 512], dt.float32,
                                             tag=f"mps{cp}{mb}", name="mps")
                          for cp in (0, 1) for mb in range(2)}
                ctiles = chase_tiles()

                # ---- pipelined gather / broadcast / modulate / main ------
                # gathers lead by 2 iterations; main matmuls trail by 2 so
                # PE never blocks the sel->drain->mul chain of the next dk.
                gts = {}
                for i in range(14):
                    if i < 12:
                        gt = gpool.tile([128, WAVE], dt.bfloat16, tag="gt",
                                        name="gt")
                        gts[i] = gt
                        for u in range(4):
                            nc.gpsimd.dma_gather(
                                gt[:, 512 * u:512 * (u + 1)].unsqueeze(1),
                                t_xT[i // 3][:],
                                idxw[w][:, i * 128 + 32 * u:
                                        i * 128 + 32 * u + 32],
                                num_idxs=512, num_idxs_reg=512, elem_size=128,
                                transpose=True, queue_num=u,
                                sbuf_tokens_per_rank=128,
                                sbuf_free_dim_per_rank=256)
                    if w == 1 and i < 12:
                        chase_phase(0, 2 + i // 6, i % 6, ctiles)
                    if i >= 2:
                        dk = i - 2
                        kb, h = dk // 2, dk % 2
                        gt = gts.pop(dk)
                        at = apool.tile([128, WAVE], dt.bfloat16, tag="at",
                                        name="at")
                        for u in range(4):
                            bps = psb.tile([128, 512], dt.float32, tag="bcps",
                                           name="bcps")
                            nc.tensor.matmul(bps[:], t_wsel[dk][:],
                                             ap_sig[:, w, u],
                                             start=True, stop=True)
                            nc.scalar.copy(at[:, u * 512:(u + 1) * 512], bps[:])
                        mt0 = mpool.tile([64, WAVE], dt.bfloat16, tag="mt0",
                                         name="mt0")
                        mt1 = mpool.tile([64, WAVE], dt.bfloat16, tag="mt1",
                                         name="mt1")
                        nc.vector.tensor_mul(mt0[:], gt[0:64, :], at[0:64, :])
                        nc.vector.tensor_mul(mt1[:], gt[64:128, :],
                                             at[64:128, :])
                        nc.vector.tensor_add(
                            shalf[kb][64 * h:64 * h + 64,
                                      w * WAVE:(w + 1) * WAVE],
                            mt0[:], mt1[:])
                    if i >= 4 and i % 2 == 0:
                        mm_step(mtiles, w, (i - 4) // 2, (0, 1))
                mm_step(mtiles, w, 5, (0, 1))
            ftiles = chase_tiles()
            for cp in (2, 3):
                for kb in range(6):
                    chase_phase(1, cp, kb, ftiles)

    nc.compile()
    return nc


# ---------------------------------------------------------------------------

def _prep_core_inputs(x, w_off, b_off, w_mask, b_mask, weight, bias, b, h):
    q0 = h * LH - HALO
    xpad = np.zeros((C, W + 1), np.float32)
    lo, hi = max(0, q0), min(L, q0 + W + 1)
    xpad[:, lo - q0:hi - q0] = x[b][:, lo:hi]
    xpad_bf = xpad.astype(bf16)

    xT = np.zeros((4, 128, WROWS), bf16)
    for d in range(4):
        rows = np.concatenate([xpad_bf[d * 64:(d + 1) * 64, :W],
                               xpad_bf[d * 64:(d + 1) * 64, 1:W + 1]],
                              axis=0).T           # [W, 128] row e
        full = np.zeros((WROWS, 128), bf16)
        full[:W] = rows
        xT[d] = full.reshape(WROWS // 128, 128, 128).transpose(1, 0, 2) \
                    .reshape(128, WROWS)
    xP = np.ascontiguousarray(xpad_bf[:, :W].reshape(2, 128, W))

    wconv = np.zeros((2, 3, 128, 64), bf16)
    for cb in range(2):
        for k in range(3):
            wconv[cb, k, :, 0:12] = w_off[:, cb * 128:(cb + 1) * 128, k].T
            wconv[cb, k, :, 32:44] = w_mask[:, cb * 128:(cb + 1) * 128, k].T
    iotas = np.zeros((2, 128, 512), np.float32)
    col = np.arange(512, dtype=np.float32)
    for t in range(2):
        for cb in range(4):
            for r in range(12):
                iotas[t, 32 * cb + r, :] = \
                    512 * (4 * t + cb) + col + (r % 3) - 1 + HALO
    boff_c = np.zeros((32, 1), np.float32)
    boff_c[0:12, 0] = b_off.astype(np.float32)
    bmask_c = np.zeros((32, 1), np.float32)
    bmask_c[0:12, 0] = b_mask.astype(np.float32)

    wmain = np.zeros((6, 2, 128, 128), bf16)
    for kb in range(6):
        for half in range(2):
            dk = 2 * kb + half
            d, k = dk // 3, dk % 3
            wblock = weight[:, d * 64:(d + 1) * 64, k]
            for mb in range(2):
                wmain[kb, mb, 64 * half:64 * half + 64, :] = \
                    wblock[mb * 128:(mb + 1) * 128, :].T
    bmain = bias.astype(np.float32).reshape(2, 128, 1)

    wsel = np.zeros((12, 24, 128), bf16)
    for dk in range(12):
        wsel[dk, dk, 0:64] = 1.0
        wsel[dk, 12 + dk, 64:128] = 1.0
    return {"xT": xT, "xP": xP, "wconv": wconv, "iotas": iotas,
            "boff": boff_c, "bmask": bmask_c,
            "wmain": wmain, "bmain": bmain, "wsel": wsel}


_CACHED = {}


def kernel(x, w_off, b_off, w_mask, b_mask, weight, bias):
    x = np.asarray(x, np.float32)
    w_off = np.asarray(w_off, np.float32)
    b_off = np.asarray(b_off, np.float32)
    w_mask = np.asarray(w_mask, np.float32)
    b_mask = np.asarray(b_mask, np.float32)
    weight = np.asarray(weight, np.float32)
    bias = np.asarray(bias, np.float32)

    if "nc" not in _CACHED:
        _CACHED["nc"] = build_program(1)
    nc = _CACHED["nc"]

    in_maps = [
        _prep_core_inputs(x, w_off, b_off, w_mask, b_mask, weight, bias,
                          core // 2, core % 2)
        for core in range(N_CORES)
    ]
    from concourse.bass_utils import run_bass_kernel_spmd
    res = run_bass_kernel_spmd(nc, in_maps, core_ids=list(range(N_CORES)))
    out = np.zeros((B, C, L), np.float32)
    for core in range(N_CORES):
        b, h = core // 2, core % 2
        y = res.results[core]["y"]
        out[b, 0:128, h * LH:(h + 1) * LH] = y[0]
        out[b, 128:256, h * LH:(h + 1) * LH] = y[1]
    return out
